# revision 14
# baseline (speedup 1.0000x reference)
"""GCN 2-layer + link decode on 8 TRN2 NeuronCores (full inputs in/out).

v3 design (dest-sharded, scatter-free, bf16, host-built sel):
- Aggregation commutes with the weight matmul: h = relu(segsum(w1*x[src]) @ W1);
  z = segsum2(w2*h[src]) @ W2.
- All tables/stages/sels bf16 (fp32 matmul is 2-pass on PE; bf16 is 1).
- Flipped routing matmul: psum[f,d] += stage[e,f].T @ sel[e,d] — the [f,d]
  psum feeds matmul(lhsT=agg[f,d], rhs=W) directly, no transposes.
- sel one-hot*weight strips are built ON HOST and DMA'd per gather window
  (DVE builds measured 507ns/desc and serialized the whole kernel in v2).
- Each core owns 12544 destination slots; edge streams sorted by
  (range, dest chunk, slot); bulk dma_gather with int16 local indices per
  32768-row range; cell sizes max-padded across cores (SPMD-uniform).
- Small gather windows (~1k rows/call) rotate across 4 SWDGE queues so the
  per-queue descriptor rings drain in parallel (desc-gen is ring-throttled).
- h AllGather'd between layers (bf16); z stays RESIDENT IN SBUF.
- Decode: pairs sharded by owner(endpoint) per side, sorted by local z slot;
  host-built sel routes SBUF z chunks into pair-chunk psums [f,pairs];
  per chunk one matmul with the Wlin half gives [pair,2]; host adds u+v.
"""
import numpy as np
import ml_dtypes

BF16 = ml_dtypes.bfloat16
P = 128
N = 100_000
NSHARD = 12_500
SLOTS = 12_544
CHUNKS = SLOTS // P          # 98
TABROWS = 8 * SLOTS          # 100352
RBOUND = [32768, 65536, 98304]
RLO = [0, 32768, 65536, 98304]
NCORES = 8
WINDOW = 4                   # chunks per gather-call window (l1/l2)
DEC_WIN = 32                 # pair-chunks per decode sel-strip window


def _range_of(a):
    return np.searchsorted(RBOUND, a, side="right")


def _wrap_idx(a):
    """[NCORES, T] int16 -> [NCORES, 128, T//16] (16-wrap, 8x replicate)."""
    ncr, t = a.shape
    out = a.reshape(ncr, t // 16, 16).transpose(0, 2, 1)
    return np.ascontiguousarray(np.tile(out, (1, 8, 1)))


def _prep_stream(tab_row, slot, w, nchunks, window):
    """SPMD-uniform gather+route stream builder (dest-major, 4 ranges).

    Returns static schedule + per-core idx16 and host-built sel strips
    sel[c] = [128, n_sel*128] bf16 with sel[pp, col*128+m] = w for the
    entry at block-position pp of desc col routing to chunk-slot m.
    """
    ncr = len(tab_row)
    counts = np.zeros((ncr, nchunks, 4), np.int64)
    for c in range(ncr):
        np.add.at(counts, (c, slot[c] // P, _range_of(tab_row[c])), 1)
    estar = counts.max(axis=0)                       # [nchunks, 4]

    layout = []
    for r in range(4):
        calls = []
        base = 0
        for k0 in range(0, nchunks, window):
            k1 = min(k0 + window, nchunks)
            cells = estar[k0:k1, r]
            offs = np.concatenate([[0], np.cumsum(cells)]).astype(np.int64)
            n = int(offs[-1])
            n_pad = max(P, ((n + P - 1) // P) * P)
            calls.append(dict(k0=k0, k1=k1, offs=offs, n=n, n_pad=n_pad,
                              base=base))
            base += n_pad
        layout.append(dict(calls=calls, T=base))

    sched = [[] for _ in range(nchunks)]
    selmap = {}
    callsel = {}                                     # (r,ci) -> (s0, s1)
    n_sel = 0
    for r in range(4):
        for ci, call in enumerate(layout[r]["calls"]):
            s0 = n_sel
            nblk = call["n_pad"] // P
            offs, k0 = call["offs"], call["k0"]
            for b in range(nblk):
                e0, e1 = b * P, b * P + P
                ks = [k for k in range(call["k0"], call["k1"])
                      if offs[k - k0] < e1 and offs[k - k0 + 1] > e0]
                if not ks:
                    ks = [call["k0"]]
                for k in ks:
                    sched[k].append(dict(r=r, call=ci, blk=b, sel=n_sel))
                    selmap[(r, ci, b, k)] = n_sel
                    n_sel += 1
            callsel[(r, ci)] = (s0, n_sel)

    idx16 = [np.zeros((ncr, layout[r]["T"]), np.int16) for r in range(4)]
    sel = [np.zeros((P, n_sel * P), BF16) for _ in range(ncr)]

    for c in range(ncr):
        tr, sl, ww = tab_row[c], slot[c], w[c]
        rr = _range_of(tr)
        ch = sl // P
        o = np.lexsort((sl, ch, rr))
        tr, sl, ww, rr, ch = tr[o], sl[o], ww[o], rr[o], ch[o]
        for r in range(4):
            m = rr == r
            if not m.any():
                continue
            trm, slm, wwm, chm = tr[m], sl[m], ww[m], ch[m]
            cell_cnt = np.zeros(nchunks, np.int64)
            np.add.at(cell_cnt, chm, 1)
            cstart = np.concatenate([[0], np.cumsum(cell_cnt)])
            within = np.arange(len(slm)) - cstart[chm]
            call_id = chm // window
            calls = layout[r]["calls"]
            cbase = np.array([cl["base"] for cl in calls], np.int64)
            cell_off = np.zeros(nchunks, np.int64)
            for ci, cl in enumerate(calls):
                for k in range(cl["k0"], cl["k1"]):
                    cell_off[k] = cl["offs"][k - cl["k0"]]
            pos = cbase[call_id] + cell_off[chm] + within
            idx16[r][c, pos] = (trm - RLO[r]).astype(np.int16)
            relpos = pos - cbase[call_id]
            blk = relpos // P
            pp = relpos % P
            cols = np.array([selmap[(r, int(ci_), int(b_), int(k_))]
                             for ci_, b_, k_ in zip(call_id, blk, chm)],
                            np.int64)
            sel[c][pp, cols * P + (slm % P)] = wwm
    return dict(layout=layout, sched=sched, n_sel=n_sel, idx16=idx16,
                sel=sel, callsel=callsel)


def _prep_decode(tt):
    """Per-core pair routing with a core-uniform slot layout.

    Pairs are sharded by owner(tt). Slot space reserves cap_j =
    max-over-cores(count of pairs whose z row is in chunk j) slots per
    z-chunk j, so the (pair-chunk k -> z-chunk j) desc schedule is
    identical on every core (SPMD). Returns per-core pair ids + slots,
    the static desc schedule, and host-built sel strips.
    """
    owner = tt // NSHARD
    loc = tt - owner * NSHARD
    ids, locs = [], []
    nj = np.zeros((NCORES, CHUNKS), np.int64)
    for c in range(NCORES):
        sel_ids = np.nonzero(owner == c)[0]
        o = np.argsort(loc[sel_ids], kind="stable")
        ids.append(sel_ids[o])
        locs.append(loc[sel_ids][o])
        np.add.at(nj, (c, locs[c] // P), 1)
    cap = nj.max(axis=0)                              # [CHUNKS]
    offs = np.concatenate([[0], np.cumsum(cap)]).astype(np.int64)
    nd = ((int(offs[-1]) + P - 1) // P) * P
    ndchunks = nd // P
    # static schedule: pair-chunk k needs z-chunk j iff slot ranges overlap
    sched = []                                        # [k] -> [j...]
    selmap = {}
    n_sel = 0
    for k in range(ndchunks):
        js = [j for j in range(CHUNKS)
              if offs[j] < (k + 1) * P and offs[j + 1] > k * P]
        if not js:
            js = [0]
        sched.append(js)
        for j in js:
            selmap[(k, j)] = n_sel
            n_sel += 1
    col0 = np.zeros(ndchunks + 1, np.int64)
    for k in range(ndchunks):
        col0[k + 1] = col0[k] + len(sched[k])
    sel = [np.zeros((P, n_sel * P), BF16) for _ in range(NCORES)]
    slots = []
    for c in range(NCORES):
        rows = locs[c]
        j_of = rows // P
        within = np.arange(len(rows)) - np.concatenate(
            [[0], np.cumsum(nj[c])])[j_of]
        sl = offs[j_of] + within
        slots.append(sl)
        k_of = sl // P
        cols = np.array([selmap[(int(k_), int(j_))]
                         for k_, j_ in zip(k_of, j_of)], np.int64)
        sel[c][rows % P, cols * P + (sl % P)] = 1.0
    return dict(ids=ids, slots=slots, nd=nd, ndchunks=ndchunks,
                sched=sched, selmap=selmap, n_sel=n_sel, sel=sel,
                col0=col0)


def kernel(x, edge_index1, edge_index2, edge_weight1, edge_weight2,
           pos_edge_index, W1, W2, Wlin):
    import concourse.bass as bass
    from concourse import bacc, tile, mybir
    from concourse.bass_utils import run_bass_kernel_spmd
    from concourse.library_config import mlp

    f32, bf16, i16 = mybir.dt.float32, mybir.dt.bfloat16, mybir.dt.int16
    i32 = mybir.dt.int32
    AF = mybir.ActivationFunctionType
    x = np.asarray(x, np.float32)
    W1 = np.asarray(W1, np.float32)
    W2 = np.asarray(W2, np.float32)
    Wlin = np.asarray(Wlin, np.float32)
    e1 = np.asarray(edge_index1).astype(np.int64)
    e2 = np.asarray(edge_index2).astype(np.int64)
    w1 = np.asarray(edge_weight1, np.float32)
    w2 = np.asarray(edge_weight2, np.float32)
    pe = np.asarray(pos_edge_index).astype(np.int64)

    # ---------- host index preprocessing ----------
    x_tab = np.zeros((TABROWS, P), BF16)
    x_tab[:N] = x.astype(BF16)
    n2row = (np.arange(N) // NSHARD) * SLOTS + (np.arange(N) % NSHARD)

    def shard_by_dest(src_rows, dst, w):
        owner = dst // NSHARD
        ld = dst - owner * NSHARD
        return ([src_rows[owner == c] for c in range(NCORES)],
                [ld[owner == c] for c in range(NCORES)],
                [w[owner == c] for c in range(NCORES)])

    l1 = _prep_stream(*shard_by_dest(e1[0], e1[1], w1), CHUNKS, WINDOW)
    l2 = _prep_stream(*shard_by_dest(n2row[e2[0]], e2[1], w2),
                      CHUNKS, WINDOW)
    du = _prep_decode(pe[0])
    dv = _prep_decode(pe[1])
    npairs = pe.shape[1]

    idx_arr = {}
    for key, pr in (("l1", l1), ("l2", l2)):
        for r in range(4):
            idx_arr[(key, r)] = _wrap_idx(pr["idx16"][r])

    # ---------- device program ----------
    nc = bacc.Bacc("TRN2", target_bir_lowering=False, debug=False,
                   num_devices=NCORES, num_swdge_queues=4,
                   dynamic_dma_scratch_size=49152)

    def din(name, shape, dt=bf16):
        return nc.dram_tensor(name, list(shape), dt, kind="ExternalInput").ap()

    xt = din("x_tab", (TABROWS, P))
    w1t = din("W1r", (P, P))
    w2t = din("W2r", (P, P))
    wab = din("Wab", (P, 4))           # W2 @ [A.T | B.T]: [f, 4]
    idx_in = {k: din(f"idx_{k[0]}_{k[1]}", v.shape[1:], i16)
              for k, v in idx_arr.items()}
    sel_in = {key: din(f"sel_{key}", (P, pr["n_sel"] * P))
              for key, pr in (("l1", l1), ("l2", l2), ("u", du), ("v", dv))}

    out_u = nc.dram_tensor("out_u", [2, P * du["ndchunks"]], f32,
                           kind="ExternalOutput").ap()
    out_v = nc.dram_tensor("out_v", [2, P * dv["ndchunks"]], f32,
                           kind="ExternalOutput").ap()
    h_slice = nc.dram_tensor("h_slice", [SLOTS, P], bf16)
    h_tab = nc.dram_tensor("h_tab", [TABROWS, P], bf16, addr_space="Shared")

    qn = [0]

    def next_q():
        qn[0] = (qn[0] + 1) % 4
        return qn[0]

    with tile.TileContext(nc) as tc:
        with (
            tc.tile_pool(name="meta", bufs=1) as mp,
            tc.tile_pool(name="stage", bufs=3) as sgp,
            tc.tile_pool(name="idxp", bufs=1) as ixp,
            tc.tile_pool(name="selp", bufs=3) as selp,
            tc.tile_pool(name="work", bufs=3) as wp,
            tc.tile_pool(name="psA", bufs=2, space="PSUM") as ppA,
            tc.tile_pool(name="psB", bufs=2, space="PSUM") as ppB,
            tc.tile_pool(name="psC", bufs=4, space="PSUM") as ppC,
        ):
            nc.gpsimd.load_library(mlp)
            w1_sb = mp.tile([P, P], bf16, name="w1_sb")
            nc.sync.dma_start(w1_sb[:], w1t[:])
            w2_sb = mp.tile([P, P], bf16, name="w2_sb")
            nc.sync.dma_start(w2_sb[:], w2t[:])
            wab_sb = mp.tile([P, 4], bf16, name="wab_sb")
            nc.sync.dma_start(wab_sb[:], wab[:])
            uv_sb = mp.tile([P, CHUNKS * 4], bf16, name="uv_sb")

            def run_agg(key, pr, tab_ap, consume):
                """Gather + sel-route; consume(k, psum[f,d]) per chunk."""
                idx_sb = []
                for r in range(4):
                    cols = pr["layout"][r]["T"] // 16
                    it = ixp.tile([P, cols], i16, name=f"ix_{key}_{r}",
                                  tag=f"ix{r}")
                    nc.sync.dma_start(it[:], idx_in[(key, r)][:])
                    idx_sb.append(it)
                stage_tiles = {}
                sel_tiles = {}

                def ensure_call(r, ci):
                    if (r, ci) in stage_tiles:
                        return
                    call = pr["layout"][r]["calls"][ci]
                    npad = call["n_pad"]
                    c0 = call["base"] // 16
                    st = sgp.tile([P, (npad // P) * P], bf16,
                                  name=f"st_{key}_{r}_{ci}", tag=f"stage{r}")
                    nc.gpsimd.dma_gather(
                        st[:].rearrange("p (c e) -> p c e", e=P),
                        tab_ap[RLO[r]:], idx_sb[r][:, c0:c0 + npad // 16],
                        npad, npad, P,
                        queue_num=next_q(), single_packet=False)
                    s0, s1 = pr["callsel"][(r, ci)]
                    se = selp.tile([P, (s1 - s0) * P], bf16,
                                   name=f"se_{key}_{r}_{ci}", tag=f"sel{r}")
                    nc.sync.dma_start(se[:], sel_in[key][:, s0 * P:s1 * P])
                    stage_tiles[(r, ci)] = st
                    sel_tiles[(r, ci)] = (se, s0)

                for k in range(CHUNKS):
                    psum_k = ppA.tile([P, P], f32, space="PSUM",
                                      name=f"ps_{key}_{k}", tag="psA")
                    descs = pr["sched"][k]
                    for j, d in enumerate(descs):
                        ensure_call(d["r"], d["call"])
                        st = stage_tiles[(d["r"], d["call"])]
                        se, s0 = sel_tiles[(d["r"], d["call"])]
                        sc = d["sel"] - s0
                        nc.tensor.matmul(
                            psum_k[:],
                            lhsT=st[:, d["blk"] * P:(d["blk"] + 1) * P],
                            rhs=se[:, sc * P:(sc + 1) * P],
                            start=(j == 0), stop=(j == len(descs) - 1))
                    consume(k, psum_k)

            def consume_l1(k, psum_k):
                agg_sb = wp.tile([P, P], bf16, name=f"a1_{k}", tag="a")
                nc.scalar.activation(agg_sb[:], psum_k[:], AF.Copy)
                h_ps = ppB.tile([P, P], f32, space="PSUM",
                                name=f"h1_{k}", tag="psB")
                nc.tensor.matmul(h_ps[:], lhsT=agg_sb[:], rhs=w1_sb[:],
                                 start=True, stop=True)
                h_sb = wp.tile([P, P], bf16, name=f"h1s_{k}", tag="h")
                nc.scalar.activation(h_sb[:], h_ps[:], AF.Relu)
                nc.sync.dma_start(h_slice[k * P:(k + 1) * P, :], h_sb[:])

            def consume_l2(k, psum_k):
                agg_sb = wp.tile([P, P], bf16, name=f"a2_{k}", tag="a")
                nc.scalar.activation(agg_sb[:], psum_k[:], AF.Copy)
                uv_ps = ppB.tile([P, 4], f32, space="PSUM",
                                 name=f"uv_{k}", tag="psB")
                nc.tensor.matmul(uv_ps[:], lhsT=agg_sb[:], rhs=wab_sb[:],
                                 start=True, stop=True)
                nc.scalar.activation(uv_sb[:, k * 4:(k + 1) * 4], uv_ps[:],
                                     AF.Copy)

            run_agg("l1", l1, xt, consume_l1)
            nc.gpsimd.collective_compute(
                "AllGather", mybir.AluOpType.bypass,
                replica_groups=[list(range(NCORES))],
                ins=[h_slice[:]], outs=[h_tab[:]])
            run_agg("l2", l2, h_tab[:], consume_l2)

            # ---- decode: host-sel routing of SBUF uv chunks ----
            def run_dec(pname, pr, wrow, out_ap):
                nwin = (pr["ndchunks"] + DEC_WIN - 1) // DEC_WIN
                for wi in range(nwin):
                    k0 = wi * DEC_WIN
                    k1 = min(k0 + DEC_WIN, pr["ndchunks"])
                    s0, s1 = int(pr["col0"][k0]), int(pr["col0"][k1])
                    se = selp.tile([P, (s1 - s0) * P], bf16,
                                   name=f"sd_{pname}_{wi}", tag="seld")
                    nc.sync.dma_start(se[:], sel_in[pname][:, s0 * P:s1 * P])
                    for k in range(k0, k1):
                        o_ps = ppC.tile([2, P], f32, space="PSUM",
                                        name=f"o_{pname}_{k}", tag="psC")
                        js = pr["sched"][k]
                        for di, j in enumerate(js):
                            col = int(pr["col0"][k]) + di
                            nc.tensor.matmul(
                                o_ps[:],
                                lhsT=uv_sb[:, j * 4 + wrow:j * 4 + wrow + 2],
                                rhs=se[:, (col - s0) * P:(col - s0 + 1) * P],
                                start=(di == 0), stop=(di == len(js) - 1))
                        o_sb = wp.tile([2, P], f32, name=f"ob_{pname}_{k}",
                                       tag="ob")
                        nc.vector.tensor_copy(o_sb[:], o_ps[:])
                        nc.sync.dma_start(
                            out_ap[:, k * P:(k + 1) * P], o_sb[:])

            run_dec("u", du, 0, out_u)
            run_dec("v", dv, 2, out_v)

    nc.compile()

    # ---------- stage inputs & run ----------
    wab_np = np.ascontiguousarray(
        W2 @ np.concatenate([Wlin[:, :P].T, Wlin[:, P:].T], axis=1)
    ).astype(BF16)
    in_maps = []
    for c in range(NCORES):
        m = {"x_tab": x_tab, "W1r": W1.astype(BF16),
             "W2r": W2.astype(BF16), "Wab": wab_np,
             "sel_l1": l1["sel"][c], "sel_l2": l2["sel"][c],
             "sel_u": du["sel"][c], "sel_v": dv["sel"][c]}
        for key in ("l1", "l2"):
            for r in range(4):
                m[f"idx_{key}_{r}"] = idx_arr[(key, r)][c]
        in_maps.append(m)

    res = run_bass_kernel_spmd(nc, in_maps, core_ids=list(range(NCORES)),
                               trace=globals().get("TRACE", False))
    globals()["LAST_EXEC_NS"] = res.exec_time_ns

    out = np.zeros((npairs, 2), np.float32)
    for pr, nm in ((du, "out_u"), (dv, "out_v")):
        for c in range(NCORES):
            o2 = res.results[c][nm]                  # [2, nd]
            sl = pr["slots"][c]
            out[pr["ids"][c]] += o2[:, sl].T
    return out
